# revision 6
# baseline (speedup 1.0000x reference)
"""Single-head attention for TRN2, 8 NeuronCores — restructured "q-route".

Problem: x [4, 2048, 1024] f32; Wq/Wk/Wv [1024, 1024]; bq/bk/bv [1024].
    out = softmax((x Wq^T + bq)(x Wk^T + bk)^T / 32) (x Wv^T + bv)

Sharding: 8 shards = (batch b, query-half h); SQ=1024 queries, SKV=2048 keys
per core; keys rotated so this core's queries come first (softmax is
permutation-invariant over keys).

Algebraic restructure (K and V projections eliminated):
    qT  = Wq xq^T + bq                    [o, s]
    A^T = Wk^T qT                         [j, s]   (bk adds a per-query
                                          constant to logits -> cancels)
    S   = A^T.T xkvT                      [s, t]
    P   = exp(S/32)  (no max subtraction; logits bounded ~8.4)
    l   = rowsum(P)  (exp accum_out)
    P^T via XBAR DMA transposes (off the PE)
    G^T = xnat ⊗ P^T                      [j, s]
    out = (G Wv^T) * (1/l) + bv           [s, o]  fp16 output, host casts f32
Score chain bf16 (rel err ~4e-3 validated), V chain fp16.
"""

import math
import os
import numpy as np

P = 128
NCH = 512

_cache = {}


def _build_program(D, SQ, SKV, n_cores, repeat=1):
    import concourse.bass as bass
    import concourse.tile as tile
    from concourse import bacc, mybir
    from contextlib import ExitStack

    f32 = mybir.dt.float32
    bf16 = mybir.dt.bfloat16
    f16 = mybir.dt.float16
    Act = mybir.ActivationFunctionType
    AX = mybir.AxisListType

    dt_ = D // P        # 8 d tiles
    sqt = SQ // P       # 8 query tiles
    skt = SKV // P      # 16 key tiles
    nsc = SQ // NCH     # 2 s-chunks
    ntc = SKV // NCH    # 4 t-chunks
    noc = D // NCH      # 2 o-chunks
    QG = 4              # query tiles per scope-C group
    ngr = sqt // QG
    scale = 1.0 / math.sqrt(D)

    nc = bacc.Bacc("TRN2", target_bir_lowering=False, debug=False,
                   num_devices=n_cores)

    wqt_d = nc.dram_tensor("wqT", [D, D], bf16, kind="ExternalInput").ap()
    wk_d = nc.dram_tensor("wk", [D, D], bf16, kind="ExternalInput").ap()
    xkvt_d = nc.dram_tensor("xkvT", [D, SKV], bf16, kind="ExternalInput").ap()
    xnat_d = nc.dram_tensor("xnat", [SKV, D], f16, kind="ExternalInput").ap()
    wvt_d = nc.dram_tensor("wvT", [D, D], f16, kind="ExternalInput").ap()
    bq_d = nc.dram_tensor("bq", [D], f32, kind="ExternalInput").ap()
    bv_d = nc.dram_tensor("bv", [D], f32, kind="ExternalInput").ap()
    out_d = nc.dram_tensor("out", [SQ, D], f16, kind="ExternalOutput").ap()

    with tile.TileContext(nc, pool_alloc_mode="queue") as tc, ExitStack() as ctx:
        const = ctx.enter_context(tc.tile_pool(name="const", bufs=1))
        bqt = const.tile([P, dt_], f32)
        nc.sync.dma_start(bqt[:], bq_d.rearrange("(t p) -> p t", p=P))
        bvb = const.tile([P, D], f32)

        for _rep in range(repeat):
            # Pools in lifetime order (released LIFO).
            xkv_pool = tc.alloc_tile_pool(name="xkvp", bufs=1)
            xkvT = [xkv_pool.tile([P, SKV], bf16, name=f"xkvT{i}",
                                  tag=f"xkvT{i}") for i in range(dt_)]
            xn_pool = tc.alloc_tile_pool(name="xnp", bufs=1)
            xnat = [xn_pool.tile([P, D], f16, name=f"xnat{i}", tag=f"xnat{i}")
                    for i in range(skt)]
            wv_pool = tc.alloc_tile_pool(name="wvp", bufs=1)
            wvT = [wv_pool.tile([P, D], f16, name=f"wvT{i}", tag=f"wvT{i}")
                   for i in range(dt_)]
            at_pool = tc.alloc_tile_pool(name="atp", bufs=1)
            At = [at_pool.tile([P, SQ], bf16, name=f"At{i}", tag=f"At{i}")
                  for i in range(dt_)]
            qt_pool = tc.alloc_tile_pool(name="qtp", bufs=1)
            qT = [qt_pool.tile([P, SQ], bf16, name=f"qT{i}", tag=f"qT{i}")
                  for i in range(dt_)]
            wk_pool = tc.alloc_tile_pool(name="wkp", bufs=1)
            wk = [wk_pool.tile([P, D], bf16, name=f"wk{i}", tag=f"wk{i}")
                  for i in range(dt_)]
            wq_pool = tc.alloc_tile_pool(name="wqp", bufs=1)
            wqT = [wq_pool.tile([P, D], bf16, name=f"wqT{i}", tag=f"wqT{i}")
                   for i in range(dt_)]

            # DMA emission order = desired arrival order: (wqT[i], xq0[i])
            # pairs feed the i-outer first pass of phase Q immediately.
            for i in range(dt_):
                nc.sync.dma_start(wqT[i][:], wqt_d[i * P:(i + 1) * P, :])
                nc.sync.dma_start(xkvT[i][:, 0:NCH],
                                  xkvt_d[i * P:(i + 1) * P, 0:NCH])
            for i in range(dt_):
                nc.sync.dma_start(xkvT[i][:, NCH:SQ],
                                  xkvt_d[i * P:(i + 1) * P, NCH:SQ])
            if _rep == 0:
                nc.gpsimd.dma_start(
                    out=bvb[:],
                    in_=bv_d.rearrange("(a d) -> a d", a=1).to_broadcast([P, D]))
            for i in range(dt_):
                nc.sync.dma_start(wk[i][:], wk_d[i * P:(i + 1) * P, :])
            for i in range(dt_):
                nc.sync.dma_start(xkvT[i][:, SQ:SKV],
                                  xkvt_d[i * P:(i + 1) * P, SQ:SKV])
            for i in range(skt):
                nc.sync.dma_start(xnat[i][:], xnat_d[i * P:(i + 1) * P, :])
            for i in range(dt_):
                nc.sync.dma_start(wvT[i][:], wvt_d[i * P:(i + 1) * P, :])

            # ---- phase Q: qT = Wq xq^T + bq   [o, s] ----------------------
            # Pass 1 (sc=0): i-outer with all 8 psum groups open, so matmuls
            # start on the first arrived wqT/xq tiles.  Pass 2 (sc=1):
            # o-outer, pipelined copies.
            psq = tc.alloc_tile_pool(name="psq", bufs=1, space="PSUM")
            ps_list = [psq.tile([P, NCH], f32, name=f"psq{o}", tag=f"psq{o}")
                       for o in range(dt_)]
            for i in range(dt_):
                for o in range(dt_):
                    nc.tensor.matmul(ps_list[o][:],
                                     wqT[i][:, o * P:(o + 1) * P],
                                     xkvT[i][:, 0:NCH],
                                     start=(i == 0), stop=(i == dt_ - 1))
            for o in range(dt_):
                if o % 2 == 0:
                    nc.scalar.activation(qT[o][:, 0:NCH], ps_list[o][:],
                                         Act.Identity, bias=bqt[:, o:o + 1])
                else:
                    nc.vector.tensor_scalar_add(qT[o][:, 0:NCH],
                                                ps_list[o][:],
                                                bqt[:, o:o + 1])
            for o in range(dt_):
                ps = psq.tile([P, NCH], f32, name=f"psq{o}b", tag=f"psq{o}")
                for i in range(dt_):
                    nc.tensor.matmul(ps[:],
                                     wqT[i][:, o * P:(o + 1) * P],
                                     xkvT[i][:, NCH:SQ],
                                     start=(i == 0), stop=(i == dt_ - 1))
                nc.scalar.activation(qT[o][:, NCH:SQ], ps[:], Act.Identity,
                                     bias=bqt[:, o:o + 1])
            psq.release()
            wq_pool.release()

            # ---- phase A: A^T = Wk^T qT   [j, s] --------------------------
            psa = tc.alloc_tile_pool(name="psa", bufs=2, space="PSUM")
            for sc in range(nsc):
                for j in range(dt_):
                    ps = psa.tile([P, NCH], f32, tag="psa")
                    for o in range(dt_):
                        nc.tensor.matmul(ps[:],
                                         wk[o][:, j * P:(j + 1) * P],
                                         qT[o][:, sc * NCH:(sc + 1) * NCH],
                                         start=(o == 0), stop=(o == dt_ - 1))
                    nc.scalar.activation(At[j][:, sc * NCH:(sc + 1) * NCH],
                                         ps[:], Act.Copy)
            psa.release()
            wk_pool.release()
            qt_pool.release()

            # ---- phase C: per group of QG query tiles ---------------------
            wc = tc.alloc_tile_pool(name="wc", bufs=1)
            pss = tc.alloc_tile_pool(name="pss", bufs=2, space="PSUM")
            psg = tc.alloc_tile_pool(name="psg", bufs=2, space="PSUM")
            pso = tc.alloc_tile_pool(name="pso", bufs=2, space="PSUM")
            for g in range(ngr):
                # strips: st[p, c, s] = P^T for the group's QG query tiles
                st = wc.tile([P, skt, QG * P], f16, tag="st", bufs=2)
                r_g = wc.tile([P, QG], f32, tag="r_g", bufs=2)
                for qq in range(QG):
                    q = g * QG + qq
                    p_t = wc.tile([P, SKV], f16, tag=f"p_t{qq}", bufs=2)
                    lpart = wc.tile([P, ntc], f32, tag=f"lp{qq}", bufs=2)
                    for tch in range(ntc):
                        ps = pss.tile([P, NCH], f32, tag="pss")
                        for j in range(dt_):
                            nc.tensor.matmul(
                                ps[:], At[j][:, q * P:(q + 1) * P],
                                xkvT[j][:, tch * NCH:(tch + 1) * NCH],
                                start=(j == 0), stop=(j == dt_ - 1))
                        nc.scalar.activation(
                            p_t[:, tch * NCH:(tch + 1) * NCH], ps[:], Act.Exp,
                            scale=scale, accum_out=lpart[:, tch:tch + 1])
                        nc.sync.dma_start_transpose(
                            st[:, tch * (NCH // P):(tch + 1) * (NCH // P),
                               qq * P:(qq + 1) * P],
                            p_t[:, tch * NCH:(tch + 1) * NCH])
                    ltot = wc.tile([P, 1], f32, tag=f"lt{qq}", bufs=2)
                    nc.vector.tensor_reduce(ltot[:], lpart[:], axis=AX.X,
                                            op=mybir.AluOpType.add)
                    nc.vector.reciprocal(r_g[:, qq:qq + 1], ltot[:])
                # G^T = xnat ⊗ strips   [j, QG*128]
                gts = []
                for j in range(dt_):
                    pg = psg.tile([P, QG * P], f32, tag="pg")
                    for c in range(skt):
                        nc.tensor.matmul(pg[:],
                                         xnat[c][:, j * P:(j + 1) * P],
                                         st[:, c, :],
                                         start=(c == 0), stop=(c == skt - 1))
                    gt = wc.tile([P, QG * P], f16, tag=f"gt{j}", bufs=1)
                    nc.scalar.activation(gt[:], pg[:], Act.Copy)
                    gts.append(gt)
                # out2 = (G Wv^T) * r + bv   [s, o]
                for qq in range(QG):
                    q = g * QG + qq
                    ot = wc.tile([P, D], f16, tag=f"ot{qq}", bufs=1)
                    for oc in range(noc):
                        po = pso.tile([P, NCH], f32, tag="pso")
                        for j in range(dt_):
                            nc.tensor.matmul(
                                po[:], gts[j][:, qq * P:(qq + 1) * P],
                                wvT[j][:, oc * NCH:(oc + 1) * NCH],
                                start=(j == 0), stop=(j == dt_ - 1))
                        nc.vector.tensor_scalar_mul(
                            ot[:, oc * NCH:(oc + 1) * NCH], po[:],
                            r_g[:, qq:qq + 1])
                        nc.vector.tensor_add(ot[:, oc * NCH:(oc + 1) * NCH],
                                             ot[:, oc * NCH:(oc + 1) * NCH],
                                             bvb[:, oc * NCH:(oc + 1) * NCH])
                        nc.sync.dma_start(
                            out_d[q * P:(q + 1) * P, oc * NCH:(oc + 1) * NCH],
                            ot[:, oc * NCH:(oc + 1) * NCH])

            pso.release()
            psg.release()
            pss.release()
            wc.release()
            at_pool.release()
            wv_pool.release()
            xn_pool.release()
            xkv_pool.release()

    nc.compile()
    return nc


def get_program(D=1024, SQ=1024, SKV=2048, n_cores=8, repeat=1):
    key = (D, SQ, SKV, n_cores, repeat)
    if key not in _cache:
        _cache[key] = _build_program(D, SQ, SKV, n_cores, repeat)
    return _cache[key]


def prep_in_maps(x, Wq, bq, Wk, bk, Wv, bv):
    """Host-side layout prep (casts/transposes/rotation only, no FLOPs)."""
    import ml_dtypes
    bf = ml_dtypes.bfloat16

    x = np.asarray(x, dtype=np.float32)
    B, S, D = x.shape
    n_cores = 8
    halves = n_cores // B
    SQ = S // halves

    wqt = np.ascontiguousarray(np.asarray(Wq, np.float32).T.astype(bf))
    wkn = np.ascontiguousarray(np.asarray(Wk, np.float32).astype(bf))
    wvt = np.ascontiguousarray(np.asarray(Wv, np.float32).T.astype(np.float16))
    bq = np.asarray(bq, dtype=np.float32)
    bv = np.asarray(bv, dtype=np.float32)

    in_maps = []
    for c in range(n_cores):
        b, h = divmod(c, halves)
        xr = np.roll(x[b], -h * SQ, axis=0)      # this core's queries first
        in_maps.append({
            "wqT": wqt, "wk": wkn, "wvT": wvt,
            "xkvT": np.ascontiguousarray(xr.T.astype(bf)),
            "xnat": np.ascontiguousarray(xr.astype(np.float16)),
            "bq": bq, "bv": bv,
        })
    return in_maps


def kernel(x, Wq, bq, Wk, bk, Wv, bv):
    from concourse.bass_utils import run_bass_kernel_spmd

    x = np.asarray(x, dtype=np.float32)
    B, S, D = x.shape
    n_cores = 8
    halves = n_cores // B
    SQ = S // halves

    nc = get_program(D=D, SQ=SQ, SKV=S, n_cores=n_cores)
    in_maps = prep_in_maps(x, Wq, bq, Wk, bk, Wv, bv)
    res = run_bass_kernel_spmd(nc, in_maps, list(range(n_cores)),
                               trace=bool(os.environ.get("ATTN_TRACE")))
    kernel.last_results = res
    out = np.stack([np.asarray(res.results[c]["out"], dtype=np.float32)
                    for c in range(n_cores)])
    return np.ascontiguousarray(
        out.reshape(B, halves, SQ, D).reshape(B, S, D))


kernel.last_results = None


# revision 7
# speedup vs baseline: 1.0892x; 1.0892x over previous
"""Single-head attention for TRN2, 8 NeuronCores — restructured "q-route".

Problem: x [4, 2048, 1024] f32; Wq/Wk/Wv [1024, 1024]; bq/bk/bv [1024].
    out = softmax((x Wq^T + bq)(x Wk^T + bk)^T / 32) (x Wv^T + bv)

Sharding: 8 shards = (batch b, query-half h); SQ=1024 queries, SKV=2048 keys
per core; keys rotated so this core's queries come first (softmax is
permutation-invariant over keys).

Algebraic restructure (K and V projections eliminated):
    qT  = Wq xq^T + bq                    [o, s]
    A^T = Wk^T qT                         [j, s]   (bk adds a per-query
                                          constant to logits -> cancels)
    S   = A^T.T xkvT                      [s, t]
    P   = exp(S/32)  (no max subtraction; logits bounded ~8.4)
    l   = rowsum(P)  (exp accum_out)
    P^T via XBAR DMA transposes (off the PE)
    G^T = xnat ⊗ P^T                      [j, s]
    out = (G Wv^T) * (1/l) + bv           [s, o]  fp16 output, host casts f32
Score chain bf16 (rel err ~4e-3 validated), V chain fp16.
"""

import math
import os
import numpy as np

P = 128
NCH = 512

_cache = {}


def _build_program(D, SQ, SKV, n_cores, repeat=1):
    import concourse.bass as bass
    import concourse.tile as tile
    from concourse import bacc, mybir
    from contextlib import ExitStack

    f32 = mybir.dt.float32
    bf16 = mybir.dt.bfloat16
    f16 = mybir.dt.float16
    Act = mybir.ActivationFunctionType
    AX = mybir.AxisListType

    dt_ = D // P        # 8 d tiles
    sqt = SQ // P       # 8 query tiles
    skt = SKV // P      # 16 key tiles
    nsc = SQ // NCH     # 2 s-chunks
    ntc = SKV // NCH    # 4 t-chunks
    noc = D // NCH      # 2 o-chunks
    QG = 4              # query tiles per scope-C group
    ngr = sqt // QG
    scale = 1.0 / math.sqrt(D)

    nc = bacc.Bacc("TRN2", target_bir_lowering=False, debug=False,
                   num_devices=n_cores)

    wqt_d = nc.dram_tensor("wqT", [D, D], bf16, kind="ExternalInput").ap()
    wk_d = nc.dram_tensor("wk", [D, D], bf16, kind="ExternalInput").ap()
    xkvt_d = nc.dram_tensor("xkvT", [D, SKV], bf16, kind="ExternalInput").ap()
    xnat_d = nc.dram_tensor("xnat", [SKV, D], f16, kind="ExternalInput").ap()
    wvt_d = nc.dram_tensor("wvT", [D, D], f16, kind="ExternalInput").ap()
    bq_d = nc.dram_tensor("bq", [D], f32, kind="ExternalInput").ap()
    bv_d = nc.dram_tensor("bv", [D], f32, kind="ExternalInput").ap()
    out_d = nc.dram_tensor("out", [SQ, D], f16, kind="ExternalOutput").ap()

    with tile.TileContext(nc, pool_alloc_mode="queue") as tc, ExitStack() as ctx:
        const = ctx.enter_context(tc.tile_pool(name="const", bufs=1))
        bqt = const.tile([P, dt_], f32)
        nc.sync.dma_start(bqt[:], bq_d.rearrange("(t p) -> p t", p=P))
        bvb = const.tile([P, D], f32)

        for _rep in range(repeat):
            # Pools in lifetime order (released LIFO).
            xkv_pool = tc.alloc_tile_pool(name="xkvp", bufs=1)
            xkvT = [xkv_pool.tile([P, SKV], bf16, name=f"xkvT{i}",
                                  tag=f"xkvT{i}") for i in range(dt_)]
            xn_pool = tc.alloc_tile_pool(name="xnp", bufs=1)
            xnat = [xn_pool.tile([P, D], f16, name=f"xnat{i}", tag=f"xnat{i}")
                    for i in range(skt)]
            wv_pool = tc.alloc_tile_pool(name="wvp", bufs=1)
            wvT = [wv_pool.tile([P, D], f16, name=f"wvT{i}", tag=f"wvT{i}")
                   for i in range(dt_)]
            at_pool = tc.alloc_tile_pool(name="atp", bufs=1)
            At = [at_pool.tile([P, SQ], bf16, name=f"At{i}", tag=f"At{i}")
                  for i in range(dt_)]
            qt_pool = tc.alloc_tile_pool(name="qtp", bufs=1)
            qT = [qt_pool.tile([P, SQ], bf16, name=f"qT{i}", tag=f"qT{i}")
                  for i in range(dt_)]
            wk_pool = tc.alloc_tile_pool(name="wkp", bufs=1)
            wk = [wk_pool.tile([P, D], bf16, name=f"wk{i}", tag=f"wk{i}")
                  for i in range(dt_)]
            wq_pool = tc.alloc_tile_pool(name="wqp", bufs=1)
            wqT = [wq_pool.tile([P, D], bf16, name=f"wqT{i}", tag=f"wqT{i}")
                   for i in range(dt_)]

            # DMA emission order = desired arrival order: (wqT[i], xq0[i])
            # pairs feed the i-outer first pass of phase Q immediately.
            for i in range(dt_):
                nc.sync.dma_start(wqT[i][:], wqt_d[i * P:(i + 1) * P, :])
                nc.sync.dma_start(xkvT[i][:, 0:NCH],
                                  xkvt_d[i * P:(i + 1) * P, 0:NCH])
            for i in range(dt_):
                nc.sync.dma_start(xkvT[i][:, NCH:SQ],
                                  xkvt_d[i * P:(i + 1) * P, NCH:SQ])
            if _rep == 0:
                nc.gpsimd.dma_start(
                    out=bvb[:],
                    in_=bv_d.rearrange("(a d) -> a d", a=1).to_broadcast([P, D]))
            for i in range(dt_):
                nc.sync.dma_start(wk[i][:], wk_d[i * P:(i + 1) * P, :])
            for i in range(dt_):
                nc.sync.dma_start(xkvT[i][:, SQ:SKV],
                                  xkvt_d[i * P:(i + 1) * P, SQ:SKV])
            for i in range(skt):
                nc.sync.dma_start(xnat[i][:], xnat_d[i * P:(i + 1) * P, :])
            for i in range(dt_):
                nc.sync.dma_start(wvT[i][:], wvt_d[i * P:(i + 1) * P, :])

            # ---- phase Q: qT = Wq xq^T + bq   [o, s] ----------------------
            # Pass 1 (sc=0): i-outer with all 8 psum groups open, so matmuls
            # start on the first arrived wqT/xq tiles.  Pass 2 (sc=1):
            # o-outer, pipelined copies.
            psq = tc.alloc_tile_pool(name="psq", bufs=1, space="PSUM")
            ps_list = [psq.tile([P, NCH], f32, name=f"psq{o}", tag=f"psq{o}")
                       for o in range(dt_)]
            for i in range(dt_):
                for o in range(dt_):
                    nc.tensor.matmul(ps_list[o][:],
                                     wqT[i][:, o * P:(o + 1) * P],
                                     xkvT[i][:, 0:NCH],
                                     start=(i == 0), stop=(i == dt_ - 1))
            for o in range(dt_):
                if o % 2 == 0:
                    nc.scalar.activation(qT[o][:, 0:NCH], ps_list[o][:],
                                         Act.Identity, bias=bqt[:, o:o + 1])
                else:
                    nc.vector.tensor_scalar_add(qT[o][:, 0:NCH],
                                                ps_list[o][:],
                                                bqt[:, o:o + 1])
            for o in range(dt_):
                ps = psq.tile([P, NCH], f32, name=f"psq{o}b", tag=f"psq{o}")
                for i in range(dt_):
                    nc.tensor.matmul(ps[:],
                                     wqT[i][:, o * P:(o + 1) * P],
                                     xkvT[i][:, NCH:SQ],
                                     start=(i == 0), stop=(i == dt_ - 1))
                nc.scalar.activation(qT[o][:, NCH:SQ], ps[:], Act.Identity,
                                     bias=bqt[:, o:o + 1])
            psq.release()
            wq_pool.release()

            # ---- phase A: A^T = Wk^T qT   [j, s] --------------------------
            psa = tc.alloc_tile_pool(name="psa", bufs=2, space="PSUM")
            for sc in range(nsc):
                for j in range(dt_):
                    ps = psa.tile([P, NCH], f32, tag="psa")
                    for o in range(dt_):
                        nc.tensor.matmul(ps[:],
                                         wk[o][:, j * P:(j + 1) * P],
                                         qT[o][:, sc * NCH:(sc + 1) * NCH],
                                         start=(o == 0), stop=(o == dt_ - 1))
                    nc.scalar.activation(At[j][:, sc * NCH:(sc + 1) * NCH],
                                         ps[:], Act.Copy)
            psa.release()
            wk_pool.release()
            qt_pool.release()

            # ---- phase C: per group of QG query tiles ---------------------
            wc = tc.alloc_tile_pool(name="wc", bufs=1)
            pss = tc.alloc_tile_pool(name="pss", bufs=2, space="PSUM")
            psg = tc.alloc_tile_pool(name="psg", bufs=2, space="PSUM")
            pso = tc.alloc_tile_pool(name="pso", bufs=2, space="PSUM")
            for g in range(ngr):
                # strips: st[p, c, s] = P^T for the group's QG query tiles
                st = wc.tile([P, skt, QG * P], f16, tag="st", bufs=2)
                r_g = wc.tile([P, QG], f32, tag="r_g", bufs=2)
                for qq in range(QG):
                    q = g * QG + qq
                    p_t = wc.tile([P, SKV], f16, tag=f"p_t{qq}", bufs=2)
                    lpart = wc.tile([P, ntc // 2], f32, tag=f"lp{qq}", bufs=2)
                    for th in range(ntc // 2):
                        ps2 = pss.tile([P, 2 * NCH], f32, tag="pss")
                        for half in range(2):
                            tch = th * 2 + half
                            for j in range(dt_):
                                nc.tensor.matmul(
                                    ps2[:, half * NCH:(half + 1) * NCH],
                                    At[j][:, q * P:(q + 1) * P],
                                    xkvT[j][:, tch * NCH:(tch + 1) * NCH],
                                    start=(j == 0), stop=(j == dt_ - 1))
                        nc.scalar.activation(
                            p_t[:, th * 2 * NCH:(th + 1) * 2 * NCH], ps2[:],
                            Act.Exp, scale=scale,
                            accum_out=lpart[:, th:th + 1])
                        nc.sync.dma_start_transpose(
                            st[:, th * (2 * NCH // P):(th + 1) * (2 * NCH // P),
                               qq * P:(qq + 1) * P],
                            p_t[:, th * 2 * NCH:(th + 1) * 2 * NCH])
                    ltot = wc.tile([P, 1], f32, tag=f"lt{qq}", bufs=2)
                    nc.vector.tensor_reduce(ltot[:], lpart[:], axis=AX.X,
                                            op=mybir.AluOpType.add)
                    nc.vector.reciprocal(r_g[:, qq:qq + 1], ltot[:])
                # G^T = xnat ⊗ strips   [j, QG*128]
                gts = []
                for j in range(dt_):
                    pg = psg.tile([P, QG * P], f32, tag="pg")
                    for c in range(skt):
                        nc.tensor.matmul(pg[:],
                                         xnat[c][:, j * P:(j + 1) * P],
                                         st[:, c, :],
                                         start=(c == 0), stop=(c == skt - 1))
                    gt = wc.tile([P, QG * P], f16, tag=f"gt{j}", bufs=1)
                    nc.scalar.activation(gt[:], pg[:], Act.Copy)
                    gts.append(gt)
                # out2 = (G Wv^T) * r + bv   [s, o]
                for qq in range(QG):
                    q = g * QG + qq
                    ot = wc.tile([P, D], f16, tag=f"ot{qq}", bufs=1)
                    for oc in range(noc):
                        po = pso.tile([P, NCH], f32, tag="pso")
                        for j in range(dt_):
                            nc.tensor.matmul(
                                po[:], gts[j][:, qq * P:(qq + 1) * P],
                                wvT[j][:, oc * NCH:(oc + 1) * NCH],
                                start=(j == 0), stop=(j == dt_ - 1))
                        nc.vector.tensor_scalar_mul(
                            ot[:, oc * NCH:(oc + 1) * NCH], po[:],
                            r_g[:, qq:qq + 1])
                        nc.vector.tensor_add(ot[:, oc * NCH:(oc + 1) * NCH],
                                             ot[:, oc * NCH:(oc + 1) * NCH],
                                             bvb[:, oc * NCH:(oc + 1) * NCH])
                    nc.sync.dma_start(out_d[q * P:(q + 1) * P, :], ot[:])

            pso.release()
            psg.release()
            pss.release()
            wc.release()
            at_pool.release()
            wv_pool.release()
            xn_pool.release()
            xkv_pool.release()

    nc.compile()
    return nc


def get_program(D=1024, SQ=1024, SKV=2048, n_cores=8, repeat=1):
    key = (D, SQ, SKV, n_cores, repeat)
    if key not in _cache:
        _cache[key] = _build_program(D, SQ, SKV, n_cores, repeat)
    return _cache[key]


def prep_in_maps(x, Wq, bq, Wk, bk, Wv, bv):
    """Host-side layout prep (casts/transposes/rotation only, no FLOPs)."""
    import ml_dtypes
    bf = ml_dtypes.bfloat16

    x = np.asarray(x, dtype=np.float32)
    B, S, D = x.shape
    n_cores = 8
    halves = n_cores // B
    SQ = S // halves

    wqt = np.ascontiguousarray(np.asarray(Wq, np.float32).T.astype(bf))
    wkn = np.ascontiguousarray(np.asarray(Wk, np.float32).astype(bf))
    wvt = np.ascontiguousarray(np.asarray(Wv, np.float32).T.astype(np.float16))
    bq = np.asarray(bq, dtype=np.float32)
    bv = np.asarray(bv, dtype=np.float32)

    in_maps = []
    for c in range(n_cores):
        b, h = divmod(c, halves)
        xr = np.roll(x[b], -h * SQ, axis=0)      # this core's queries first
        in_maps.append({
            "wqT": wqt, "wk": wkn, "wvT": wvt,
            "xkvT": np.ascontiguousarray(xr.T.astype(bf)),
            "xnat": np.ascontiguousarray(xr.astype(np.float16)),
            "bq": bq, "bv": bv,
        })
    return in_maps


def kernel(x, Wq, bq, Wk, bk, Wv, bv):
    from concourse.bass_utils import run_bass_kernel_spmd

    x = np.asarray(x, dtype=np.float32)
    B, S, D = x.shape
    n_cores = 8
    halves = n_cores // B
    SQ = S // halves

    nc = get_program(D=D, SQ=SQ, SKV=S, n_cores=n_cores)
    in_maps = prep_in_maps(x, Wq, bq, Wk, bk, Wv, bv)
    res = run_bass_kernel_spmd(nc, in_maps, list(range(n_cores)),
                               trace=bool(os.environ.get("ATTN_TRACE")))
    kernel.last_results = res
    out = np.stack([np.asarray(res.results[c]["out"], dtype=np.float32)
                    for c in range(n_cores)])
    return np.ascontiguousarray(
        out.reshape(B, halves, SQ, D).reshape(B, S, D))


kernel.last_results = None


# revision 10
# speedup vs baseline: 1.1421x; 1.0486x over previous
"""Single-head attention for TRN2, 8 NeuronCores — restructured "q-route".

Problem: x [4, 2048, 1024] f32; Wq/Wk/Wv [1024, 1024]; bq/bk/bv [1024].
    out = softmax((x Wq^T + bq)(x Wk^T + bk)^T / 32) (x Wv^T + bv)

Sharding: 8 shards = (batch b, query-half h); SQ=1024 queries, SKV=2048 keys
per core; keys rotated so this core's queries come first (softmax is
permutation-invariant over keys).

Algebraic restructure (K and V projections eliminated):
    qT  = Wq xq^T + bq                    [o, s]
    A^T = Wk^T qT                         [j, s]   (bk adds a per-query
                                          constant to logits -> cancels)
    S   = A^T.T xkvT                      [s, t]
    P   = exp(S/32)  (no max subtraction; logits bounded ~8.4)
    l   = rowsum(P)  (exp accum_out)
    P^T via XBAR DMA transposes (off the PE)
    G^T = xnat ⊗ P^T                      [j, s]
    out = (G Wv^T) * (1/l) + bv           [s, o]  fp16 output, host casts f32
Score chain bf16 (rel err ~4e-3 validated), V chain fp16.
"""

import math
import os
import numpy as np

P = 128
NCH = 512

_cache = {}


def _build_program(D, SQ, SKV, n_cores, repeat=1):
    import concourse.bass as bass
    import concourse.tile as tile
    from concourse import bacc, mybir
    from contextlib import ExitStack

    f32 = mybir.dt.float32
    bf16 = mybir.dt.bfloat16
    f16 = mybir.dt.float16
    Act = mybir.ActivationFunctionType
    AX = mybir.AxisListType

    dt_ = D // P        # 8 d tiles
    sqt = SQ // P       # 8 query tiles
    skt = SKV // P      # 16 key tiles
    nsc = SQ // NCH     # 2 s-chunks
    ntc = SKV // NCH    # 4 t-chunks
    noc = D // NCH      # 2 o-chunks
    QG = 4              # query tiles per scope-C group
    ngr = sqt // QG
    scale = 1.0 / math.sqrt(D)

    nc = bacc.Bacc("TRN2", target_bir_lowering=False, debug=False,
                   num_devices=n_cores)

    wqt_d = nc.dram_tensor("wqT", [D, D], bf16, kind="ExternalInput").ap()
    wk_d = nc.dram_tensor("wk", [D, D], bf16, kind="ExternalInput").ap()
    xkvt_d = nc.dram_tensor("xkvT", [D, SKV], bf16, kind="ExternalInput").ap()
    xnat_d = nc.dram_tensor("xnat", [SKV, D], f16, kind="ExternalInput").ap()
    wvt_d = nc.dram_tensor("wvT", [D, D], f16, kind="ExternalInput").ap()
    bq_d = nc.dram_tensor("bq", [D], f32, kind="ExternalInput").ap()
    bv_d = nc.dram_tensor("bv", [D], f32, kind="ExternalInput").ap()
    out_d = nc.dram_tensor("out", [SQ, D], f16, kind="ExternalOutput").ap()

    with tile.TileContext(nc, pool_alloc_mode="queue") as tc, ExitStack() as ctx:
        const = ctx.enter_context(tc.tile_pool(name="const", bufs=1))
        bqt = const.tile([P, dt_], f32)
        nc.sync.dma_start(bqt[:], bq_d.rearrange("(t p) -> p t", p=P))
        bvb = const.tile([P, D], f32)

        for _rep in range(repeat):
            # Pools in lifetime order (released LIFO).
            xkv_pool = tc.alloc_tile_pool(name="xkvp", bufs=1)
            xkvT = [xkv_pool.tile([P, SKV], bf16, name=f"xkvT{i}",
                                  tag=f"xkvT{i}") for i in range(dt_)]
            xn_pool = tc.alloc_tile_pool(name="xnp", bufs=1)
            xnat = [xn_pool.tile([P, D], f16, name=f"xnat{i}", tag=f"xnat{i}")
                    for i in range(skt)]
            wv_pool = tc.alloc_tile_pool(name="wvp", bufs=1)
            wvT = [wv_pool.tile([P, D], f16, name=f"wvT{i}", tag=f"wvT{i}")
                   for i in range(dt_)]
            at_pool = tc.alloc_tile_pool(name="atp", bufs=1)
            At = [at_pool.tile([P, SQ], bf16, name=f"At{i}", tag=f"At{i}")
                  for i in range(dt_)]
            qt_pool = tc.alloc_tile_pool(name="qtp", bufs=1)
            qT = [qt_pool.tile([P, SQ], bf16, name=f"qT{i}", tag=f"qT{i}")
                  for i in range(dt_)]
            wk_pool = tc.alloc_tile_pool(name="wkp", bufs=1)
            wk = [wk_pool.tile([P, D], bf16, name=f"wk{i}", tag=f"wk{i}")
                  for i in range(dt_)]
            wq_pool = tc.alloc_tile_pool(name="wqp", bufs=1)
            wqT = [wq_pool.tile([P, D], bf16, name=f"wqT{i}", tag=f"wqT{i}")
                   for i in range(dt_)]

            # DMA emission order = desired arrival order: (wqT[i], xq0[i])
            # pairs feed the i-outer first pass of phase Q immediately.
            for i in range(dt_):
                nc.sync.dma_start(wqT[i][:], wqt_d[i * P:(i + 1) * P, :])
                nc.sync.dma_start(xkvT[i][:, 0:NCH],
                                  xkvt_d[i * P:(i + 1) * P, 0:NCH])
            for i in range(dt_):
                nc.sync.dma_start(xkvT[i][:, NCH:SQ],
                                  xkvt_d[i * P:(i + 1) * P, NCH:SQ])
            if _rep == 0:
                nc.gpsimd.dma_start(
                    out=bvb[:],
                    in_=bv_d.rearrange("(a d) -> a d", a=1).to_broadcast([P, D]))
            for i in range(dt_):
                nc.sync.dma_start(wk[i][:], wk_d[i * P:(i + 1) * P, :])
            for i in range(dt_):
                nc.sync.dma_start(xkvT[i][:, SQ:SKV],
                                  xkvt_d[i * P:(i + 1) * P, SQ:SKV])
            for i in range(skt):
                nc.sync.dma_start(xnat[i][:], xnat_d[i * P:(i + 1) * P, :])
            for i in range(dt_):
                nc.sync.dma_start(wvT[i][:], wvt_d[i * P:(i + 1) * P, :])

            # ---- phase Q: qT = Wq xq^T + bq   [o, s] ----------------------
            # Pass 1 (sc=0): i-outer with all 8 psum groups open, so matmuls
            # start on the first arrived wqT/xq tiles.  Pass 2 (sc=1):
            # o-outer, pipelined copies.
            psq = tc.alloc_tile_pool(name="psq", bufs=1, space="PSUM")
            ps_list = [psq.tile([P, NCH], f32, name=f"psq{o}", tag=f"psq{o}")
                       for o in range(dt_)]
            for i in range(dt_):
                for o in range(dt_):
                    nc.tensor.matmul(ps_list[o][:],
                                     wqT[i][:, o * P:(o + 1) * P],
                                     xkvT[i][:, 0:NCH],
                                     start=(i == 0), stop=(i == dt_ - 1))
            for o in range(dt_):
                if o % 2 == 0:
                    nc.scalar.activation(qT[o][:, 0:NCH], ps_list[o][:],
                                         Act.Identity, bias=bqt[:, o:o + 1])
                else:
                    nc.vector.tensor_scalar_add(qT[o][:, 0:NCH],
                                                ps_list[o][:],
                                                bqt[:, o:o + 1])
            for o in range(dt_):
                ps = psq.tile([P, NCH], f32, name=f"psq{o}b", tag=f"psq{o}")
                for i in range(dt_):
                    nc.tensor.matmul(ps[:],
                                     wqT[i][:, o * P:(o + 1) * P],
                                     xkvT[i][:, NCH:SQ],
                                     start=(i == 0), stop=(i == dt_ - 1))
                nc.scalar.activation(qT[o][:, NCH:SQ], ps[:], Act.Identity,
                                     bias=bqt[:, o:o + 1])
            psq.release()
            wq_pool.release()

            # ---- phase A: A^T = Wk^T qT   [j, s] --------------------------
            psa = tc.alloc_tile_pool(name="psa", bufs=2, space="PSUM")
            for sc in range(nsc):
                for j in range(dt_):
                    ps = psa.tile([P, NCH], f32, tag="psa")
                    for o in range(dt_):
                        nc.tensor.matmul(ps[:],
                                         wk[o][:, j * P:(j + 1) * P],
                                         qT[o][:, sc * NCH:(sc + 1) * NCH],
                                         start=(o == 0), stop=(o == dt_ - 1))
                    nc.scalar.activation(At[j][:, sc * NCH:(sc + 1) * NCH],
                                         ps[:], Act.Copy)
            psa.release()
            wk_pool.release()
            qt_pool.release()

            # ---- phase C: per group of QG query tiles ---------------------
            wc = tc.alloc_tile_pool(name="wc", bufs=1)
            pss = tc.alloc_tile_pool(name="pss", bufs=2, space="PSUM")
            psg = tc.alloc_tile_pool(name="psg", bufs=2, space="PSUM")
            pso = tc.alloc_tile_pool(name="pso", bufs=2, space="PSUM")
            def _emit_out2(g, gts, r_g):
                for qq in range(QG):
                    q = g * QG + qq
                    ot = wc.tile([P, D], f16, tag=f"ot{qq}", bufs=1)
                    for oc in range(noc):
                        po = pso.tile([P, NCH], f32, tag="pso")
                        for j in range(dt_):
                            nc.tensor.matmul(
                                po[:], gts[j][:, qq * P:(qq + 1) * P],
                                wvT[j][:, oc * NCH:(oc + 1) * NCH],
                                start=(j == 0), stop=(j == dt_ - 1))
                        nc.vector.tensor_scalar_mul(
                            ot[:, oc * NCH:(oc + 1) * NCH], po[:],
                            r_g[:, qq:qq + 1])
                        nc.vector.tensor_add(ot[:, oc * NCH:(oc + 1) * NCH],
                                             ot[:, oc * NCH:(oc + 1) * NCH],
                                             bvb[:, oc * NCH:(oc + 1) * NCH])
                    nc.sync.dma_start(out_d[q * P:(q + 1) * P, :], ot[:])

            pending = None
            for g in range(ngr):
                # strips: st[p, c, s] = P^T for the group's QG query tiles
                st = wc.tile([P, skt, QG * P], f16, tag="st", bufs=2)
                r_g = wc.tile([P, QG], f32, tag="r_g", bufs=2)
                for qq in range(QG):
                    q = g * QG + qq
                    p_t = wc.tile([P, SKV], f16, tag=f"p_t{qq}", bufs=2)
                    lpart = wc.tile([P, ntc // 2], f32, tag=f"lp{qq}", bufs=2)
                    for th in range(ntc // 2):
                        ps2 = pss.tile([P, 2 * NCH], f32, tag="pss")
                        for half in range(2):
                            tch = th * 2 + half
                            for j in range(dt_):
                                nc.tensor.matmul(
                                    ps2[:, half * NCH:(half + 1) * NCH],
                                    At[j][:, q * P:(q + 1) * P],
                                    xkvT[j][:, tch * NCH:(tch + 1) * NCH],
                                    start=(j == 0), stop=(j == dt_ - 1))
                        nc.scalar.activation(
                            p_t[:, th * 2 * NCH:(th + 1) * 2 * NCH], ps2[:],
                            Act.Exp, scale=scale,
                            accum_out=lpart[:, th:th + 1])
                        nc.sync.dma_start_transpose(
                            st[:, th * (2 * NCH // P):(th + 1) * (2 * NCH // P),
                               qq * P:(qq + 1) * P],
                            p_t[:, th * 2 * NCH:(th + 1) * 2 * NCH])
                    ltot = wc.tile([P, 1], f32, tag=f"lt{qq}", bufs=2)
                    nc.vector.tensor_reduce(ltot[:], lpart[:], axis=AX.X,
                                            op=mybir.AluOpType.add)
                    nc.vector.reciprocal(r_g[:, qq:qq + 1], ltot[:])
                # out2 of the PREVIOUS group runs here: its matmuls hide
                # the exp/transpose latency of this group's strips.
                if pending is not None:
                    _emit_out2(*pending)
                # G^T = xnat ⊗ strips   [j, QG*128]
                gts = []
                for j in range(dt_):
                    pg = psg.tile([P, QG * P], f32, tag="pg")
                    for c in range(skt):
                        nc.tensor.matmul(pg[:],
                                         xnat[c][:, j * P:(j + 1) * P],
                                         st[:, c, :],
                                         start=(c == 0), stop=(c == skt - 1))
                    gt = wc.tile([P, QG * P], f16, tag=f"gt{j}", bufs=2)
                    nc.scalar.activation(gt[:], pg[:], Act.Copy)
                    gts.append(gt)
                pending = (g, gts, r_g)

            if pending is not None:
                _emit_out2(*pending)
            pso.release()
            psg.release()
            pss.release()
            wc.release()
            at_pool.release()
            wv_pool.release()
            xn_pool.release()
            xkv_pool.release()

    nc.compile()
    return nc


def get_program(D=1024, SQ=1024, SKV=2048, n_cores=8, repeat=1):
    key = (D, SQ, SKV, n_cores, repeat)
    if key not in _cache:
        _cache[key] = _build_program(D, SQ, SKV, n_cores, repeat)
    return _cache[key]


def prep_in_maps(x, Wq, bq, Wk, bk, Wv, bv):
    """Host-side layout prep (casts/transposes/rotation only, no FLOPs)."""
    import ml_dtypes
    bf = ml_dtypes.bfloat16

    x = np.asarray(x, dtype=np.float32)
    B, S, D = x.shape
    n_cores = 8
    halves = n_cores // B
    SQ = S // halves

    wqt = np.ascontiguousarray(np.asarray(Wq, np.float32).T.astype(bf))
    wkn = np.ascontiguousarray(np.asarray(Wk, np.float32).astype(bf))
    wvt = np.ascontiguousarray(np.asarray(Wv, np.float32).T.astype(np.float16))
    bq = np.asarray(bq, dtype=np.float32)
    bv = np.asarray(bv, dtype=np.float32)

    in_maps = []
    for c in range(n_cores):
        b, h = divmod(c, halves)
        xr = np.roll(x[b], -h * SQ, axis=0)      # this core's queries first
        in_maps.append({
            "wqT": wqt, "wk": wkn, "wvT": wvt,
            "xkvT": np.ascontiguousarray(xr.T.astype(bf)),
            "xnat": np.ascontiguousarray(xr.astype(np.float16)),
            "bq": bq, "bv": bv,
        })
    return in_maps


def kernel(x, Wq, bq, Wk, bk, Wv, bv):
    from concourse.bass_utils import run_bass_kernel_spmd

    x = np.asarray(x, dtype=np.float32)
    B, S, D = x.shape
    n_cores = 8
    halves = n_cores // B
    SQ = S // halves

    nc = get_program(D=D, SQ=SQ, SKV=S, n_cores=n_cores)
    in_maps = prep_in_maps(x, Wq, bq, Wk, bk, Wv, bv)
    res = run_bass_kernel_spmd(nc, in_maps, list(range(n_cores)),
                               trace=bool(os.environ.get("ATTN_TRACE")))
    kernel.last_results = res
    out = np.stack([np.asarray(res.results[c]["out"], dtype=np.float32)
                    for c in range(n_cores)])
    return np.ascontiguousarray(
        out.reshape(B, halves, SQ, D).reshape(B, S, D))


kernel.last_results = None
